# revision 49
# baseline (speedup 1.0000x reference)
"""TRN2 Bass kernel for nn_AttentionStoreProcessor (dense transformer attention).

Full (unsharded) inputs in, full output out. Internally:
  - 20 heads = 8 cores x (2 full heads + 1 half-query head). SPMD-uniform
    program: odd cores see the token axis rolled by 1024 (attention is
    permutation-equivariant over keys; CAPE frame weights are swapped on the
    host so each local chunk uses its real frame).
  - CAPE rotation + softmax scale folded into Wq/Wk on host.
  - hs arrives bf16; hsT built by XBAR DMA-transpose (no PE transposes).
  - Projections in bf16; q/k re-quantized to fp8e4 and DMA-rearranged into
    the [32, 2, S] DoubleRow layout; scores run as fp8 DoubleRow matmuls
    (2 contraction tiles/instr at 0.5 cyc/row).
  - exp split between ACT (true exp -> bf16) and DVE (Schraudolph: one
    tensor_scalar to int16, bitcast to bf16).
  - PV in bf16 with a ones-column in v for the softmax sums; normalize via
    DVE reciprocal + PE broadcast matmul; per-head outputs written as fp8.
  - Output projection as fp8 DoubleRow (contraction 2x128 in one instr);
    out DMA'd as bf16; host adds bias + residual and un-rolls odd cores.
"""
import numpy as np
import ml_dtypes
from contextlib import ExitStack

import concourse.bacc as bacc
import concourse.mybir as mybir
import concourse.tile as tile
from concourse.bass_utils import run_bass_kernel_spmd

F32 = mybir.dt.float32
F32R = mybir.dt.float32r
BF16 = mybir.dt.bfloat16
FP8 = mybir.dt.float8e4
I16 = mybir.dt.int16
AF = mybir.ActivationFunctionType
ALU = mybir.AluOpType
MPM = mybir.MatmulPerfMode

HEADS = 20
N_CORES = 8
S = 2048
D = 1280
HD = 64
L = 1024
KT = D // 128  # 10 contraction tiles for projections
TOKT = S // 128  # 16 token tiles

A_SCH = 128.0 / np.log(2.0)
B_SCH = 127.0 * 128.0 - 5.5

_CACHED_NC = None


def _build_nc():
    nc = bacc.Bacc("TRN2", debug=False, num_devices=N_CORES)

    hst = nc.dram_tensor("hst", [128, KT * S], BF16, kind="ExternalInput").ap()
    wg = nc.dram_tensor("wg", [128, 6 * KT * 128], BF16, kind="ExternalInput").ap()
    wv = nc.dram_tensor("wv", [128, KT * 192], FP8, kind="ExternalInput").ap()
    hst8 = nc.dram_tensor("hst8", [128, KT * S], FP8, kind="ExternalInput").ap()
    wo8d = nc.dram_tensor("wo8", [128, 2, D], FP8, kind="ExternalInput").ap()
    out = nc.dram_tensor("out", [S, D], FP8, kind="ExternalOutput").ap()
    out_r = out.rearrange("(n p) d -> n p d", p=128)

    with (
        tile.TileContext(nc) as tc,
        ExitStack() as ctx,
        nc.allow_low_precision(reason="bf16/fp8 used deliberately; tol is 2e-2"),
    ):
        persist = ctx.enter_context(tc.tile_pool(name="persist", bufs=1))
        v195 = persist.tile([128, TOKT, 195], BF16, tag="v195")
        outT = persist.tile([128, 2, S], FP8, tag="outT")
        oT1 = persist.tile([64, S], FP8, tag="oT1")
        ones = persist.tile([65, 64], F32, tag="ones")
        wo8 = persist.tile([128, 2, D], FP8, tag="wo8")
        qd = [
            persist.tile([32, 2, S], FP8, tag="qd0", name="qd0"),
            persist.tile([32, 2, S], FP8, tag="qd1", name="qd1"),
            persist.tile([32, 2, L], FP8, tag="qd2", name="qd2"),
        ]
        kd = [
            persist.tile([32, 2, S], FP8, tag="kd0", name="kd0"),
            persist.tile([32, 2, S], FP8, tag="kd1", name="kd1"),
            persist.tile([32, 2, S], FP8, tag="kd2", name="kd2"),
        ]

        # constants / zero fills (DVE is idle at startup)
        nc.vector.memset(ones[64:65, :], 1.0)
        nc.vector.memset(outT[:, 1, :], 0.0)
        for h in range(3):
            nc.vector.memset(v195[:, :, 65 * h + 64], 1.0)

        # ---- phase 1: load + transpose + projections ----
        s1 = tc.alloc_tile_pool(name="s1", bufs=1)
        wg_sb = s1.tile([128, 6 * KT * 128], BF16, tag="wg")
        wv_sb = s1.tile([128, KT * 192], FP8, tag="wv")
        hsT8 = s1.tile([128, KT, S], FP8, tag="hsT8", name="hsT8")
        hsT = s1.tile([128, KT, S], BF16, tag="hsT", name="hsT")
        qk8 = [s1.tile([128, S], FP8, tag=f"qk8{g}", name=f"qk8{g}") for g in range(3)]

        # weights on the ACT hwdge queue; transposes on the SP queue.
        # Half-token-slab XBAR transposes: 20 DMA instructions (HWDGE overhead
        # per instruction is ~625ns, so fewer/bigger is better), and the
        # first 10 unblock projection chunks 0-1 early.
        half_w = 3 * KT * 128
        nc.sync.dma_start(wg_sb[:, 0:half_w], wg[:, 0:half_w])
        for kt in range(KT):
            nc.sync.dma_start(hsT[:, kt, :], hst[:, kt * S : (kt + 1) * S])
        nc.sync.dma_start(wg_sb[:, half_w:], wg[:, half_w:])
        nc.sync.dma_start(wv_sb[:], wv)
        nc.sync.dma_start(wo8[:], wo8d)
        nc.sync.dma_start(hsT8[:], hst8)

        pp_ps = tc.alloc_tile_pool(name="pp", bufs=6, space="PSUM")
        vp_ps = tc.alloc_tile_pool(name="vp", bufs=2, space="PSUM")

        def qk_pair(ch_pair):
            # 6 psum groups accumulate concurrently, kt-outer: PE stays busy
            # from the first arriving hsT slab
            pps = {}
            for ch in ch_pair:
                for g in range(3):
                    pps[(ch, g)] = pp_ps.tile(
                        [128, 512], F32, tag="pp", name=f"pp{ch}_{g}"
                    )
            for kt in range(KT):
                for ch in ch_pair:
                    t = ch // 2
                    qs = slice(ch * 512, (ch + 1) * 512)
                    for g in range(3):
                        base = ((t * 3 + g) * KT) * 128
                        nc.tensor.matmul(
                            pps[(ch, g)][:],
                            wg_sb[:, base + kt * 128 : base + (kt + 1) * 128],
                            hsT[:, kt, qs],
                            start=(kt == 0),
                            stop=(kt == KT - 1),
                        )
            for ch in ch_pair:
                qs = slice(ch * 512, (ch + 1) * 512)
                for g in range(3):
                    nc.vector.tensor_copy(qk8[g][:, qs], pps[(ch, g)][:])

        def v_tiles(lo, hi):
            for n in range(lo, hi):
                ts = slice(n * 128, (n + 1) * 128)
                vp = vp_ps.tile([128, 192], F32, tag="vp", name=f"vp{n}")
                for j in range(KT // 2):
                    rhs = wv_sb[:, j * 384 : (j + 1) * 384].rearrange(
                        "p (i f) -> p i f", i=2
                    )
                    nc.tensor.matmul(
                        vp[:],
                        hsT8[:, 2 * j : 2 * j + 2, ts],
                        rhs,
                        start=(j == 0),
                        stop=(j == KT // 2 - 1),
                        perf_mode=MPM.DoubleRow,
                    )
                src_ap = vp[:].rearrange("p (h x) -> p h x", h=3)
                dst = v195[:, n, :].rearrange("p (h x) -> p h x", h=3)[:, :, 0:64]
                if n % 2 == 0:
                    nc.scalar.copy(dst, src_ap)
                else:
                    nc.vector.tensor_copy(dst, src_ap)

        qk_pair((0, 1))
        qk_pair((2, 3))

        # q/k fp8 rearrange into DoubleRow layout [32, 2, *]; these DMAs
        # overlap the V projection matmuls below
        for i in range(2):
            r0, r1 = 32 * i, 32 * (i + 1)
            nc.sync.dma_start(qd[0][:, i, :], qk8[0][r0:r1, :])
            nc.sync.dma_start(qd[1][:, i, :], qk8[0][64 + r0 : 64 + r1, :])
            nc.sync.dma_start(kd[0][:, i, :], qk8[1][r0:r1, :])
            nc.sync.dma_start(kd[1][:, i, :], qk8[1][64 + r0 : 64 + r1, :])
            nc.sync.dma_start(qd[2][:, i, :], qk8[2][r0:r1, 0:L])
            nc.sync.dma_start(kd[2][:, i, :], qk8[2][64 + r0 : 64 + r1, :])

        v_tiles(0, TOKT)

        vp_ps.release()
        pp_ps.release()
        s1.release()

        def dr_score(dst, h, kt, qoff, w):
            nc.tensor.matmul(
                dst,
                kd[h][:, :, kt * 128 : (kt + 1) * 128],
                qd[h][:, :, qoff : qoff + w],
                start=True,
                stop=True,
                perf_mode=MPM.DoubleRow,
            )

        def pv_mm(pv_t, col, h, kt, u_rhs):
            nc.tensor.matmul(
                pv_t[:, col : col + 512],
                v195[:, kt, 65 * h : 65 * h + 65],
                u_rhs,
                start=(kt == 0),
                stop=(kt == TOKT - 1),
            )

        # ---- phase 2: attention, one head-pass at a time ----
        # 512-query-block units for pipeline depth: sc [128,512] bufs=4
        # (4 banks) + pvL/pvR [65,512] bufs=2 each (4 banks) = 8 banks.
        # The normalize broadcast matmul borrows an sc slot at pass end.
        sc_ps = tc.alloc_tile_pool(name="sc", bufs=4, space="PSUM")
        pvl_ps = tc.alloc_tile_pool(name="pvl", bufs=2, space="PSUM")
        pvr_ps = tc.alloc_tile_pool(name="pvr", bufs=2, space="PSUM")
        ua_pool = tc.alloc_tile_pool(name="ua", bufs=6)
        ui_pool = tc.alloc_tile_pool(name="ui", bufs=6)
        rc_pool = tc.alloc_tile_pool(name="rc", bufs=2)
        bcs_pool = tc.alloc_tile_pool(name="bcs", bufs=2)

        def normalize(h, pv_halves, qcol, nm, pi=None):
            rc = rc_pool.tile([65, 1024], F32R, tag="rc", name=f"rc{nm}")
            bcs = bcs_pool.tile([64, 1024], F32, tag="bcs", name=f"bcs{nm}")
            if h == 0:
                dest = outT[0:64, 0, qcol]
            elif h == 1:
                dest = oT1[:, qcol]
            else:
                dest = outT[0:64, 1, qcol]
            bc = sc_ps.tile([128, 512], F32, tag="sc", name=f"bc{nm}")
            for s_ in range(2):
                cs = slice(s_ * 512, (s_ + 1) * 512)
                nc.vector.reciprocal(rc[64:65, cs], pv_halves[s_][64:65, :])
                nc.tensor.matmul(
                    bc[0:64, :],
                    ones[64:65, :].bitcast(F32R),
                    rc[64:65, cs],
                    start=True,
                    stop=True,
                )
                nc.scalar.copy(bcs[:, cs], bc[0:64, :])
            for s_ in range(2):
                cs = slice(s_ * 512, (s_ + 1) * 512)
                nc.vector.tensor_mul(dest[:, cs], pv_halves[s_][0:64, :], bcs[:, cs])
            if h == 1:
                nc.sync.dma_start(outT[64:128, 0, qcol], oT1[:, qcol])

        # pass list: (head, local query offset); h2 covers local 0:1024 only
        passes = [(0, 0), (1, 0), (2, 0), (0, 1024), (1, 1024)]
        pending_norm = None
        for pi, (h, q0) in enumerate(passes):
            pvh = [
                pvl_ps.tile([65, 512], F32, tag="pvl", name=f"pvl{pi}"),
                pvr_ps.tile([65, 512], F32, tag="pvr", name=f"pvr{pi}"),
            ]
            for kt in range(TOKT):
                if kt == 5 and pending_norm is not None:
                    normalize(*pending_norm)
                    pending_norm = None
                for blk in range(2):
                    sc = sc_ps.tile(
                        [128, 512], F32, tag="sc", name=f"sc{pi}_{kt}_{blk}"
                    )
                    dr_score(sc[:], h, kt, q0 + blk * 512, 512)
                    # blk0 -> ACT, blk1 -> DVE, with every 8th blk1 on ACT
                    if blk == 0 or kt % 8 == 7:
                        u = ua_pool.tile(
                            [128, 512], BF16, tag="ua", name=f"u{pi}_{kt}_{blk}"
                        )
                        nc.scalar.activation(u[:], sc[:], AF.Exp)
                        urhs = u[:]
                    else:
                        u = ui_pool.tile(
                            [128, 512], I16, tag="ui", name=f"u{pi}_{kt}_{blk}"
                        )
                        nc.vector.tensor_scalar(
                            u[:], sc[:], A_SCH, B_SCH, ALU.mult, ALU.add
                        )
                        urhs = u[:].bitcast(BF16)
                    nc.tensor.matmul(
                        pvh[blk][:],
                        v195[:, kt, 65 * h : 65 * h + 65],
                        urhs,
                        start=(kt == 0),
                        stop=(kt == TOKT - 1),
                    )
            qcol = slice(q0, q0 + 1024)
            pending_norm = (h, pvh, qcol, f"p{pi}", pi)
        normalize(*pending_norm)

        bcs_pool.release()
        rc_pool.release()
        ui_pool.release()
        ua_pool.release()
        pvr_ps.release()
        pvl_ps.release()
        sc_ps.release()

        # ---- output projection (fp8 DoubleRow) ----
        op_ps = tc.alloc_tile_pool(name="op", bufs=8, space="PSUM")
        ob_pool = tc.alloc_tile_pool(name="ob", bufs=8)
        out_r4 = out.rearrange("(m n p) d -> m p n d", n=2, p=128)
        for m in range(8):
            ob = ob_pool.tile([128, 2, D], FP8, tag="ob", name=f"ob{m}")
            for j in range(2):
                n = m * 2 + j
                ts = slice(n * 128, (n + 1) * 128)
                ops = []
                for dc, (off, w) in enumerate(((0, 512), (512, 512), (1024, 256))):
                    op = op_ps.tile([128, 512], F32, tag="op", name=f"op{n}_{dc}")
                    nc.tensor.matmul(
                        op[:, 0:w],
                        outT[:, :, ts],
                        wo8[:, :, off : off + w],
                        start=True,
                        stop=True,
                        perf_mode=MPM.DoubleRow,
                    )
                    ops.append((op, off, w))
                eng = nc.vector.tensor_copy if n % 2 == 0 else nc.scalar.copy
                for op, off, w in ops:
                    eng(ob[:, j, off : off + w], op[:, 0:w])
            nc.sync.dma_start(out_r4[m], ob[:])
        ob_pool.release()
        op_ps.release()

    nc.compile()
    return nc


def _get_nc():
    global _CACHED_NC
    if _CACHED_NC is None:
        _CACHED_NC = _build_nc()
    return _CACHED_NC


def _fold_cape(W, P):
    """W @ blockdiag(P) for 4x4 P repeated along channels: exact CAPE fold."""
    d = W.shape[1]
    W4 = W.reshape(W.shape[0], d // 4, 4)
    return np.einsum("cik,kj->cij", W4, P, optimize=True).reshape(W.shape[0], d)


def _klayout(W):
    """[1280, C] -> [128, KT*C] with ktile-major free dim."""
    C = W.shape[1]
    return np.ascontiguousarray(
        W.reshape(KT, 128, C).transpose(1, 0, 2).reshape(128, KT * C)
    )


def _prep_in_maps(hidden_states, p_out, p_out_inv, Wq, Wk, Wv, Wo):
    scale = HD ** -0.5
    hs2 = np.ascontiguousarray(hidden_states.reshape(S, D), dtype=np.float32)
    hsb = hs2.astype(ml_dtypes.bfloat16)

    Wq_eff = [(_fold_cape(Wq, p_out_inv[0, t]) * scale).astype(np.float32) for t in range(2)]
    Wk_eff = [_fold_cape(Wk, p_out[0, t]).astype(np.float32) for t in range(2)]

    def cols(W, h):
        return W[:, h * HD : (h + 1) * HD]

    in_maps = []
    for c in range(N_CORES):
        p = c // 2
        if c % 2 == 0:
            fa, fb = 5 * p, 5 * p + 1
        else:
            fa, fb = 5 * p + 2, 5 * p + 3
        fc = 5 * p + 4
        roll = (c % 2) * L
        hs_c = np.roll(hsb, -roll, axis=0) if roll else hsb
        hst_c = np.ascontiguousarray(
            hs_c.T.reshape(KT, 128, S).transpose(1, 0, 2).reshape(128, KT * S)
        )
        frames = (0, 1) if c % 2 == 0 else (1, 0)
        blocks = []
        for t_real in frames:
            G0 = np.concatenate([cols(Wq_eff[t_real], fa), cols(Wq_eff[t_real], fb)], 1)
            G1 = np.concatenate([cols(Wk_eff[t_real], fa), cols(Wk_eff[t_real], fb)], 1)
            G2 = np.concatenate([cols(Wq_eff[t_real], fc), cols(Wk_eff[t_real], fc)], 1)
            blocks += [_klayout(G0), _klayout(G1), _klayout(G2)]
        wg_c = np.concatenate(blocks, axis=1).astype(ml_dtypes.bfloat16)
        wv_cols = np.concatenate([cols(Wv, fa), cols(Wv, fb), cols(Wv, fc)], 1)
        wv_c = np.ascontiguousarray(
            wv_cols.reshape(KT // 2, 2, 128, 192)
            .transpose(2, 0, 1, 3)
            .reshape(128, KT * 192)
        ).astype(ml_dtypes.float8_e4m3fn)
        tile0 = np.concatenate([Wo[fa * HD : (fa + 1) * HD], Wo[fb * HD : (fb + 1) * HD]], 0)
        tile1 = np.concatenate([Wo[fc * HD : (fc + 1) * HD], np.zeros((64, D), np.float32)], 0)
        wo8_c = np.ascontiguousarray(
            np.stack([tile0, tile1], axis=1)
        ).astype(ml_dtypes.float8_e4m3fn)
        in_maps.append(
            {
                "hst": hst_c,
                "hst8": hst_c.astype(ml_dtypes.float8_e4m3fn),
                "wg": wg_c,
                "wv": wv_c,
                "wo8": wo8_c,
            }
        )
    return in_maps


def kernel(hidden_states, p_out, p_out_inv, Wq, Wk, Wv, Wo, bo):
    hidden_states = np.asarray(hidden_states, dtype=np.float32)
    in_maps = _prep_in_maps(
        hidden_states,
        np.asarray(p_out, np.float32),
        np.asarray(p_out_inv, np.float32),
        np.asarray(Wq, np.float32),
        np.asarray(Wk, np.float32),
        np.asarray(Wv, np.float32),
        np.asarray(Wo, np.float32),
    )
    nc = _get_nc()
    res = run_bass_kernel_spmd(nc, in_maps, core_ids=list(range(N_CORES)))
    acc = np.zeros((S, D), np.float32)
    for c in range(N_CORES):
        o = res.results[c]["out"].astype(np.float32)  # fp8 -> f32
        roll = (c % 2) * L
        acc += np.roll(o, roll, axis=0) if roll else o
    acc += np.asarray(bo, np.float32)[None, :]
    out = acc.reshape(2, L, D) + hidden_states
    return out


# revision 50
# speedup vs baseline: 1.0026x; 1.0026x over previous
"""TRN2 Bass kernel for nn_AttentionStoreProcessor (dense transformer attention).

Full (unsharded) inputs in, full output out. Internally:
  - 20 heads = 8 cores x (2 full heads + 1 half-query head). SPMD-uniform
    program: odd cores see the token axis rolled by 1024 (attention is
    permutation-equivariant over keys; CAPE frame weights are swapped on the
    host so each local chunk uses its real frame).
  - CAPE rotation + softmax scale folded into Wq/Wk on host.
  - hs arrives bf16; hsT built by XBAR DMA-transpose (no PE transposes).
  - Projections in bf16; q/k re-quantized to fp8e4 and DMA-rearranged into
    the [32, 2, S] DoubleRow layout; scores run as fp8 DoubleRow matmuls
    (2 contraction tiles/instr at 0.5 cyc/row).
  - exp split between ACT (true exp -> bf16) and DVE (Schraudolph: one
    tensor_scalar to int16, bitcast to bf16).
  - PV in bf16 with a ones-column in v for the softmax sums; normalize via
    DVE reciprocal + PE broadcast matmul; per-head outputs written as fp8.
  - Output projection as fp8 DoubleRow (contraction 2x128 in one instr);
    out DMA'd as bf16; host adds bias + residual and un-rolls odd cores.
"""
import numpy as np
import ml_dtypes
from contextlib import ExitStack

import concourse.bacc as bacc
import concourse.mybir as mybir
import concourse.tile as tile
from concourse.bass_utils import run_bass_kernel_spmd

F32 = mybir.dt.float32
F32R = mybir.dt.float32r
BF16 = mybir.dt.bfloat16
FP8 = mybir.dt.float8e4
I16 = mybir.dt.int16
AF = mybir.ActivationFunctionType
ALU = mybir.AluOpType
MPM = mybir.MatmulPerfMode

HEADS = 20
N_CORES = 8
S = 2048
D = 1280
HD = 64
L = 1024
KT = D // 128  # 10 contraction tiles for projections
TOKT = S // 128  # 16 token tiles

A_SCH = 128.0 / np.log(2.0)
B_SCH = 127.0 * 128.0 - 5.5

_CACHED_NC = None


def _build_nc():
    nc = bacc.Bacc("TRN2", debug=False, num_devices=N_CORES)

    hst = nc.dram_tensor("hst", [128, KT * S], BF16, kind="ExternalInput").ap()
    wg = nc.dram_tensor("wg", [128, 6 * KT * 128], BF16, kind="ExternalInput").ap()
    wv = nc.dram_tensor("wv", [128, KT * 192], FP8, kind="ExternalInput").ap()
    hst8 = nc.dram_tensor("hst8", [128, KT * S], FP8, kind="ExternalInput").ap()
    wo8d = nc.dram_tensor("wo8", [128, 2, D], FP8, kind="ExternalInput").ap()
    out = nc.dram_tensor("out", [S, D], FP8, kind="ExternalOutput").ap()
    out_r = out.rearrange("(n p) d -> n p d", p=128)

    with (
        tile.TileContext(nc) as tc,
        ExitStack() as ctx,
        nc.allow_low_precision(reason="bf16/fp8 used deliberately; tol is 2e-2"),
    ):
        persist = ctx.enter_context(tc.tile_pool(name="persist", bufs=1))
        v195 = persist.tile([128, TOKT, 195], BF16, tag="v195")
        outT = persist.tile([128, 2, S], FP8, tag="outT")
        oT1 = persist.tile([64, S], FP8, tag="oT1")
        ones = persist.tile([65, 64], F32, tag="ones")
        wo8 = persist.tile([128, 2, D], FP8, tag="wo8")
        qd = [
            persist.tile([32, 2, S], FP8, tag="qd0", name="qd0"),
            persist.tile([32, 2, S], FP8, tag="qd1", name="qd1"),
            persist.tile([32, 2, L], FP8, tag="qd2", name="qd2"),
        ]
        kd = [
            persist.tile([32, 2, S], FP8, tag="kd0", name="kd0"),
            persist.tile([32, 2, S], FP8, tag="kd1", name="kd1"),
            persist.tile([32, 2, S], FP8, tag="kd2", name="kd2"),
        ]

        # constants / zero fills (DVE is idle at startup)
        nc.vector.memset(ones[64:65, :], 1.0)
        nc.vector.memset(outT[:, 1, :], 0.0)
        for h in range(3):
            nc.vector.memset(v195[:, :, 65 * h + 64], 1.0)

        # ---- phase 1: load + transpose + projections ----
        s1 = tc.alloc_tile_pool(name="s1", bufs=1)
        wg_sb = s1.tile([128, 6 * KT * 128], BF16, tag="wg")
        wv_sb = s1.tile([128, KT * 192], FP8, tag="wv")
        hsT8 = s1.tile([128, KT, S], FP8, tag="hsT8", name="hsT8")
        hsT = s1.tile([128, KT, S], BF16, tag="hsT", name="hsT")
        qk8 = [s1.tile([128, S], FP8, tag=f"qk8{g}", name=f"qk8{g}") for g in range(3)]

        # weights on the ACT hwdge queue; transposes on the SP queue.
        # Half-token-slab XBAR transposes: 20 DMA instructions (HWDGE overhead
        # per instruction is ~625ns, so fewer/bigger is better), and the
        # first 10 unblock projection chunks 0-1 early.
        half_w = 3 * KT * 128
        nc.sync.dma_start(wg_sb[:, 0:half_w], wg[:, 0:half_w])
        for kt in range(KT):
            nc.sync.dma_start(hsT[:, kt, :], hst[:, kt * S : (kt + 1) * S])
        nc.sync.dma_start(wg_sb[:, half_w:], wg[:, half_w:])
        nc.sync.dma_start(wv_sb[:], wv)
        nc.sync.dma_start(wo8[:], wo8d)
        nc.sync.dma_start(hsT8[:], hst8)

        pp_ps = tc.alloc_tile_pool(name="pp", bufs=6, space="PSUM")
        vp_ps = tc.alloc_tile_pool(name="vp", bufs=2, space="PSUM")

        def qk_pair(ch_pair):
            # 6 psum groups accumulate concurrently, kt-outer: PE stays busy
            # from the first arriving hsT slab
            pps = {}
            for ch in ch_pair:
                for g in range(3):
                    pps[(ch, g)] = pp_ps.tile(
                        [128, 512], F32, tag="pp", name=f"pp{ch}_{g}"
                    )
            for kt in range(KT):
                for ch in ch_pair:
                    t = ch // 2
                    qs = slice(ch * 512, (ch + 1) * 512)
                    for g in range(3):
                        base = ((t * 3 + g) * KT) * 128
                        nc.tensor.matmul(
                            pps[(ch, g)][:],
                            wg_sb[:, base + kt * 128 : base + (kt + 1) * 128],
                            hsT[:, kt, qs],
                            start=(kt == 0),
                            stop=(kt == KT - 1),
                        )
            for ch in ch_pair:
                qs = slice(ch * 512, (ch + 1) * 512)
                for g in range(3):
                    nc.vector.tensor_copy(qk8[g][:, qs], pps[(ch, g)][:])

        def v_tiles(lo, hi):
            for n in range(lo, hi):
                ts = slice(n * 128, (n + 1) * 128)
                vp = vp_ps.tile([128, 192], F32, tag="vp", name=f"vp{n}")
                for j in range(KT // 2):
                    rhs = wv_sb[:, j * 384 : (j + 1) * 384].rearrange(
                        "p (i f) -> p i f", i=2
                    )
                    nc.tensor.matmul(
                        vp[:],
                        hsT8[:, 2 * j : 2 * j + 2, ts],
                        rhs,
                        start=(j == 0),
                        stop=(j == KT // 2 - 1),
                        perf_mode=MPM.DoubleRow,
                    )
                src_ap = vp[:].rearrange("p (h x) -> p h x", h=3)
                dst = v195[:, n, :].rearrange("p (h x) -> p h x", h=3)[:, :, 0:64]
                if n % 2 == 0:
                    nc.scalar.copy(dst, src_ap)
                else:
                    nc.vector.tensor_copy(dst, src_ap)

        qk_pair((0, 1))
        qk_pair((2, 3))

        # q/k fp8 rearrange into DoubleRow layout [32, 2, *]; these DMAs
        # overlap the V projection matmuls below
        for i in range(2):
            r0, r1 = 32 * i, 32 * (i + 1)
            nc.sync.dma_start(qd[0][:, i, :], qk8[0][r0:r1, :])
            nc.sync.dma_start(qd[1][:, i, :], qk8[0][64 + r0 : 64 + r1, :])
            nc.sync.dma_start(kd[0][:, i, :], qk8[1][r0:r1, :])
            nc.sync.dma_start(kd[1][:, i, :], qk8[1][64 + r0 : 64 + r1, :])
            nc.sync.dma_start(qd[2][:, i, :], qk8[2][r0:r1, 0:L])
            nc.sync.dma_start(kd[2][:, i, :], qk8[2][64 + r0 : 64 + r1, :])

        v_tiles(0, TOKT)

        vp_ps.release()
        pp_ps.release()
        s1.release()

        def dr_score(dst, h, kt, qoff, w):
            nc.tensor.matmul(
                dst,
                kd[h][:, :, kt * 128 : (kt + 1) * 128],
                qd[h][:, :, qoff : qoff + w],
                start=True,
                stop=True,
                perf_mode=MPM.DoubleRow,
            )

        def pv_mm(pv_t, col, h, kt, u_rhs):
            nc.tensor.matmul(
                pv_t[:, col : col + 512],
                v195[:, kt, 65 * h : 65 * h + 65],
                u_rhs,
                start=(kt == 0),
                stop=(kt == TOKT - 1),
            )

        # ---- phase 2: attention, one head-pass at a time ----
        # 512-query-block units for pipeline depth: sc [128,512] bufs=4
        # (4 banks) + pvL/pvR [65,512] bufs=2 each (4 banks) = 8 banks.
        # The normalize broadcast matmul borrows an sc slot at pass end.
        sc_ps = tc.alloc_tile_pool(name="sc", bufs=4, space="PSUM")
        pvl_ps = tc.alloc_tile_pool(name="pvl", bufs=2, space="PSUM")
        pvr_ps = tc.alloc_tile_pool(name="pvr", bufs=2, space="PSUM")
        ua_pool = tc.alloc_tile_pool(name="ua", bufs=6)
        ui_pool = tc.alloc_tile_pool(name="ui", bufs=6)
        rc_pool = tc.alloc_tile_pool(name="rc", bufs=2)
        bcs_pool = tc.alloc_tile_pool(name="bcs", bufs=2)

        def normalize(h, pv_halves, qcol, nm, pi=None):
            rc = rc_pool.tile([65, 1024], F32R, tag="rc", name=f"rc{nm}")
            bcs = bcs_pool.tile([64, 1024], F32, tag="bcs", name=f"bcs{nm}")
            if h == 0:
                dest = outT[0:64, 0, qcol]
            elif h == 1:
                dest = oT1[:, qcol]
            else:
                dest = outT[0:64, 1, qcol]
            bc = sc_ps.tile([128, 512], F32, tag="sc", name=f"bc{nm}")
            for s_ in range(2):
                cs = slice(s_ * 512, (s_ + 1) * 512)
                nc.vector.reciprocal(rc[64:65, cs], pv_halves[s_][64:65, :])
                nc.tensor.matmul(
                    bc[0:64, :],
                    ones[64:65, :].bitcast(F32R),
                    rc[64:65, cs],
                    start=True,
                    stop=True,
                )
                nc.scalar.copy(bcs[:, cs], bc[0:64, :])
            for s_ in range(2):
                cs = slice(s_ * 512, (s_ + 1) * 512)
                nc.vector.tensor_mul(dest[:, cs], pv_halves[s_][0:64, :], bcs[:, cs])
            if h == 1:
                nc.sync.dma_start(outT[64:128, 0, qcol], oT1[:, qcol])

        # pass list: (head, local query offset); h2 covers local 0:1024 only
        passes = [(0, 0), (1, 0), (2, 0), (0, 1024), (1, 1024)]
        pending_norm = None
        for pi, (h, q0) in enumerate(passes):
            pvh = [
                pvl_ps.tile([65, 512], F32, tag="pvl", name=f"pvl{pi}"),
                pvr_ps.tile([65, 512], F32, tag="pvr", name=f"pvr{pi}"),
            ]
            for kt in range(TOKT):
                if kt == 5 and pending_norm is not None:
                    normalize(*pending_norm)
                    pending_norm = None
                for blk in range(2):
                    sc = sc_ps.tile(
                        [128, 512], F32, tag="sc", name=f"sc{pi}_{kt}_{blk}"
                    )
                    dr_score(sc[:], h, kt, q0 + blk * 512, 512)
                    # blk0 -> ACT, blk1 -> DVE, with every 8th blk1 on ACT
                    if blk == 0 or kt % 8 == 7:
                        u = ua_pool.tile(
                            [128, 512], BF16, tag="ua", name=f"u{pi}_{kt}_{blk}"
                        )
                        nc.scalar.activation(u[:], sc[:], AF.Exp)
                        urhs = u[:]
                    else:
                        u = ui_pool.tile(
                            [128, 512], I16, tag="ui", name=f"u{pi}_{kt}_{blk}"
                        )
                        nc.vector.tensor_scalar(
                            u[:], sc[:], A_SCH, B_SCH, ALU.mult, ALU.add
                        )
                        urhs = u[:].bitcast(BF16)
                    nc.tensor.matmul(
                        pvh[blk][:],
                        v195[:, kt, 65 * h : 65 * h + 65],
                        urhs,
                        start=(kt == 0),
                        stop=(kt == TOKT - 1),
                    )
            qcol = slice(q0, q0 + 1024)
            pending_norm = (h, pvh, qcol, f"p{pi}", pi)
        normalize(*pending_norm)

        bcs_pool.release()
        rc_pool.release()
        ui_pool.release()
        ua_pool.release()
        pvr_ps.release()
        pvl_ps.release()
        sc_ps.release()

        # ---- output projection (fp8 DoubleRow) ----
        op_ps = tc.alloc_tile_pool(name="op", bufs=6, space="PSUM")
        ob_pool = tc.alloc_tile_pool(name="ob", bufs=8)
        out_r4 = out.rearrange("(m n p) d -> m p n d", n=2, p=128)
        for m in range(8):
            ob = ob_pool.tile([128, 2, D], FP8, tag="ob", name=f"ob{m}")
            for j in range(2):
                n = m * 2 + j
                ts = slice(n * 128, (n + 1) * 128)
                ops = []
                for dc, (off, w) in enumerate(((0, 512), (512, 512), (1024, 256))):
                    op = op_ps.tile([128, 512], F32, tag="op", name=f"op{n}_{dc}")
                    nc.tensor.matmul(
                        op[:, 0:w],
                        outT[:, :, ts],
                        wo8[:, :, off : off + w],
                        start=True,
                        stop=True,
                        perf_mode=MPM.DoubleRow,
                    )
                    ops.append((op, off, w))
                eng = nc.vector.tensor_copy if n % 2 == 0 else nc.scalar.copy
                for op, off, w in ops:
                    eng(ob[:, j, off : off + w], op[:, 0:w])
            nc.sync.dma_start(out_r4[m], ob[:])
        ob_pool.release()
        op_ps.release()

    nc.compile()
    return nc


def _get_nc():
    global _CACHED_NC
    if _CACHED_NC is None:
        _CACHED_NC = _build_nc()
    return _CACHED_NC


def _fold_cape(W, P):
    """W @ blockdiag(P) for 4x4 P repeated along channels: exact CAPE fold."""
    d = W.shape[1]
    W4 = W.reshape(W.shape[0], d // 4, 4)
    return np.einsum("cik,kj->cij", W4, P, optimize=True).reshape(W.shape[0], d)


def _klayout(W):
    """[1280, C] -> [128, KT*C] with ktile-major free dim."""
    C = W.shape[1]
    return np.ascontiguousarray(
        W.reshape(KT, 128, C).transpose(1, 0, 2).reshape(128, KT * C)
    )


def _prep_in_maps(hidden_states, p_out, p_out_inv, Wq, Wk, Wv, Wo):
    scale = HD ** -0.5
    hs2 = np.ascontiguousarray(hidden_states.reshape(S, D), dtype=np.float32)
    hsb = hs2.astype(ml_dtypes.bfloat16)

    Wq_eff = [(_fold_cape(Wq, p_out_inv[0, t]) * scale).astype(np.float32) for t in range(2)]
    Wk_eff = [_fold_cape(Wk, p_out[0, t]).astype(np.float32) for t in range(2)]

    def cols(W, h):
        return W[:, h * HD : (h + 1) * HD]

    in_maps = []
    for c in range(N_CORES):
        p = c // 2
        if c % 2 == 0:
            fa, fb = 5 * p, 5 * p + 1
        else:
            fa, fb = 5 * p + 2, 5 * p + 3
        fc = 5 * p + 4
        roll = (c % 2) * L
        hs_c = np.roll(hsb, -roll, axis=0) if roll else hsb
        hst_c = np.ascontiguousarray(
            hs_c.T.reshape(KT, 128, S).transpose(1, 0, 2).reshape(128, KT * S)
        )
        frames = (0, 1) if c % 2 == 0 else (1, 0)
        blocks = []
        for t_real in frames:
            G0 = np.concatenate([cols(Wq_eff[t_real], fa), cols(Wq_eff[t_real], fb)], 1)
            G1 = np.concatenate([cols(Wk_eff[t_real], fa), cols(Wk_eff[t_real], fb)], 1)
            G2 = np.concatenate([cols(Wq_eff[t_real], fc), cols(Wk_eff[t_real], fc)], 1)
            blocks += [_klayout(G0), _klayout(G1), _klayout(G2)]
        wg_c = np.concatenate(blocks, axis=1).astype(ml_dtypes.bfloat16)
        wv_cols = np.concatenate([cols(Wv, fa), cols(Wv, fb), cols(Wv, fc)], 1)
        wv_c = np.ascontiguousarray(
            wv_cols.reshape(KT // 2, 2, 128, 192)
            .transpose(2, 0, 1, 3)
            .reshape(128, KT * 192)
        ).astype(ml_dtypes.float8_e4m3fn)
        tile0 = np.concatenate([Wo[fa * HD : (fa + 1) * HD], Wo[fb * HD : (fb + 1) * HD]], 0)
        tile1 = np.concatenate([Wo[fc * HD : (fc + 1) * HD], np.zeros((64, D), np.float32)], 0)
        wo8_c = np.ascontiguousarray(
            np.stack([tile0, tile1], axis=1)
        ).astype(ml_dtypes.float8_e4m3fn)
        in_maps.append(
            {
                "hst": hst_c,
                "hst8": hst_c.astype(ml_dtypes.float8_e4m3fn),
                "wg": wg_c,
                "wv": wv_c,
                "wo8": wo8_c,
            }
        )
    return in_maps


def kernel(hidden_states, p_out, p_out_inv, Wq, Wk, Wv, Wo, bo):
    hidden_states = np.asarray(hidden_states, dtype=np.float32)
    in_maps = _prep_in_maps(
        hidden_states,
        np.asarray(p_out, np.float32),
        np.asarray(p_out_inv, np.float32),
        np.asarray(Wq, np.float32),
        np.asarray(Wk, np.float32),
        np.asarray(Wv, np.float32),
        np.asarray(Wo, np.float32),
    )
    nc = _get_nc()
    res = run_bass_kernel_spmd(nc, in_maps, core_ids=list(range(N_CORES)))
    acc = np.zeros((S, D), np.float32)
    for c in range(N_CORES):
        o = res.results[c]["out"].astype(np.float32)  # fp8 -> f32
        roll = (c % 2) * L
        acc += np.roll(o, roll, axis=0) if roll else o
    acc += np.asarray(bo, np.float32)[None, :]
    out = acc.reshape(2, L, D) + hidden_states
    return out
